# revision 69
# baseline (speedup 1.0000x reference)
"""Causal multi-head attention block (qkv proj + partial RoPE + causal attn +
out proj) for Trainium2, distributed over 8 NeuronCores.

Sharding: core i handles batch b = i//2 and head-group g = i%2 (6 of 12 heads).
Each core computes a partial output projection (contraction over its 6 heads'
384 channels); the host sums the two head-group partials per batch.

v4 design notes (evolved from the v2 baseline at 397us; now ~253us):
  - All matmul operands bf16 (rel err ~3.3e-3 vs the 2e-2 budget); weight
    tiles padded to 128 columns so Fast Weight Load stays on.
  - Everything runs in the PE's full 128x128 mode -- no tile_position row
    packing anywhere.  Mode switches drain the PE, so the v2-style 64-row
    score pairs cost more than they saved.  Scores use zero-padded q copies
    (qz_h0 = [q_h0 | 0], qz_h1 = [0 | q_h1]) against the full 128-row k
    block: one LDWEIGHTS per k-tile, K=128, no mode switch.
  - No scatter DMAs for the qk layout: within each head the dims are
    permuted (pass d16:64 at rows 0:48, rope'd d0:16 at rows 48:64; scores
    are invariant to a consistent q/k dim permutation).  Each per-pair-block
    projection M-tile computes the pass rows in place (wqk columns zeroed at
    the rope rows), the rotated rope rows are accumulated into the same
    PSUM tile by one-hot routing matmuls (rmats), and a single DVE copy
    evicts the finished 128-row block to qk_sb.
  - Causal mask: GpSimd tri-multiply on the exp output (diag tiles only);
    the GpSimd is otherwise idle and the attn@v consumer runs a full k-tile
    later (lag-1 emission), so the mask is off the critical path.
  - Softmax: ones-column in v gives rowsums in o_ps row 64; per-pair DMA
    gather of the two rowsum rows, reciprocal_approx_fast (the accurate DVE
    reciprocal costs 3.3us per call at 2 partitions), K=2 bf16 broadcast
    matmul, DVE multiply into o_sb.
  - Emission schedule (single in-order stream per engine): proj(0), then
    for each jt: attn(jt) with proj(jt+1) and outproj(jt-1) generators
    interleaved unit-by-unit into its stream.  This keeps the PE busy while
    the ACT paces the exps, eliminates the phase alternation, and avoids
    the PE cold-clock (half-rate) state that follows any idle gap.
"""

import numpy as np

B, T, C = 4, 2048, 768
NH, HD, RD = 12, 64, 16
NHL = NH // 2          # heads per core (local)
NPAIR = NHL // 2       # head pairs per core
CL = NHL * HD          # local channels (384)
TQ = 512               # q tile
NTQ = T // TQ
NKT = T // 128         # k tiles of 128

_cache = {}


def _build(debug=False):
    import concourse.bacc as bacc
    import concourse.mybir as mybir
    import concourse.tile as tile

    F32R = mybir.dt.float32r
    F32 = mybir.dt.float32
    BF16 = mybir.dt.bfloat16
    AF = mybir.ActivationFunctionType
    MUL = mybir.AluOpType.mult
    SUB = mybir.AluOpType.subtract
    ADD = mybir.AluOpType.add

    nc = bacc.Bacc(trn_type="TRN2", name="attn8")

    xt = nc.dram_tensor("xt", [C, T], BF16, kind="ExternalInput")
    wqkt = nc.dram_tensor("wqkt", [C, 1024], BF16, kind="ExternalInput")
    wvt = nc.dram_tensor("wvt", [C, CL], BF16, kind="ExternalInput")
    wot = nc.dram_tensor("wot", [CL, C], BF16, kind="ExternalInput")
    cosb = nc.dram_tensor("cosb", [96, T], BF16, kind="ExternalInput")
    sinb = nc.dram_tensor("sinb", [96, T], BF16, kind="ExternalInput")
    tri = nc.dram_tensor("tri", [128, 2 * 128], BF16, kind="ExternalInput")
    rmats = nc.dram_tensor("rmats", [96, 12 * 128], BF16, kind="ExternalInput")
    e2 = nc.dram_tensor("e2", [2, 128], BF16, kind="ExternalInput")
    out = nc.dram_tensor("out", [C, T], F32, kind="ExternalOutput")
    if debug:
        dbg_qk = nc.dram_tensor("dbg_qk", [128, 2 * NPAIR, T], F32,
                                kind="ExternalOutput")
        dbg_v = nc.dram_tensor("dbg_v", [128, NKT, NHL, 128], F32,
                               kind="ExternalOutput")
        dbg_o = nc.dram_tensor("dbg_o", [128, NPAIR, T], F32,
                               kind="ExternalOutput")

    with tile.TileContext(nc) as tc:
        with (
            tc.tile_pool(name="persist", bufs=1) as pp,
            tc.tile_pool(name="weights", bufs=1) as wp,
            tc.tile_pool(name="xload", bufs=2) as xlp,
            tc.tile_pool(name="ropet", bufs=1) as rtp,
            tc.tile_pool(name="expp", bufs=5) as xpp,
            tc.tile_pool(name="ostg", bufs=2) as osg,
            tc.tile_pool(name="onorm", bufs=4) as onp,
            tc.tile_pool(name="flex", bufs=2, space="PSUM") as flx,
            tc.tile_pool(name="sps", bufs=2, space="PSUM") as sps,
            tc.tile_pool(name="ops", bufs=1, space="PSUM") as ops,
        ):
            qk_sb = pp.tile([128, 2 * NPAIR, T], BF16, tag="qk")
            v_sb = pp.tile([128, NKT, NHL, 128], BF16, tag="v")
            o_sb = pp.tile([128, NPAIR, T], BF16, tag="o")
            cos_t = pp.tile([96, T], BF16, tag="cos")
            sin_t = pp.tile([96, T], BF16, tag="sin")
            tri_t = pp.tile([128, 2, 128], BF16, tag="tri")
            rm_t = pp.tile([96, 12 * 128], BF16, tag="rmats")
            e2_t = pp.tile([2, 128], BF16, tag="e2")
            rot1 = pp.tile([96, T], BF16, tag="rot1")
            rot2 = pp.tile([96, T], BF16, tag="rot2")
            qz0 = pp.tile([128, 2, TQ], BF16, tag="qz0")
            qz1 = pp.tile([128, 2, TQ], BF16, tag="qz1")

            wqk_t = wp.tile([128, C // 128, 1024], BF16, tag="wqk")
            wv_t = wp.tile([128, C // 128, CL], BF16, tag="wv")
            wo_t = wp.tile([128, NPAIR, C], BF16, tag="wo")


            # ---- weights / tables (first-needed first) ----
            wqk_r = wqkt.rearrange("(co p) m -> co p m", p=128)
            wv_r = wvt.rearrange("(co p) m -> co p m", p=128)
            x_r = xt.rearrange("(co p) t -> p co t", p=128)
            x_tiles = {}

            def emit_x(jt, split=False):
                x_jt = xlp.tile([128, C // 128, TQ], BF16, tag="x")
                ts = slice(jt * TQ, (jt + 1) * TQ)
                if split:  # per-c DMAs so the first proj chain starts early
                    for c in range(C // 128):
                        nc.sync.dma_start(x_jt[:, c], x_r[:, c, ts])
                else:
                    nc.sync.dma_start(x_jt, x_r[:, :, ts])
                x_tiles[jt] = x_jt

            for c in range(C // 128):
                nc.sync.dma_start(wqk_t[:, c, 0:256], wqk_r[c, :, 0:256])
            emit_x(0, split=True)
            nc.sync.dma_start(cos_t, cosb[:, :])
            nc.sync.dma_start(sin_t, sinb[:, :])
            for c in range(C // 128):
                nc.sync.dma_start(wqk_t[:, c, 256:1024], wqk_r[c, :, 256:1024])
            nc.sync.dma_start(rm_t, rmats[:, :])
            nc.gpsimd.memset(
                v_sb.bitcast(mybir.dt.uint16).rearrange("p a b c -> p (a b c)"),
                0x3F80)  # bf16 1.0 bit pattern
            nc.gpsimd.memset(
                qz0.bitcast(mybir.dt.uint16).rearrange("p a b -> p (a b)"), 0)
            nc.gpsimd.memset(
                qz1.bitcast(mybir.dt.uint16).rearrange("p a b -> p (a b)"), 0)
            for c in range(C // 128):
                nc.sync.dma_start(wv_t[:, c], wv_r[c])
            nc.sync.dma_start(
                tri_t.rearrange("p a b -> p (a b)"), tri[:, :])
            nc.sync.dma_start(e2_t, e2[:, :])
            nc.sync.dma_start(
                wo_t, wot.rearrange("(po p) m -> p po m", p=128))

            def gen_proj(jt):
                """Generator: emits the qkv projection for t-tile jt in ~2-3
                matmul units per step so it can be interleaved into the
                previous tile's attention stream."""
                ts = slice(jt * TQ, (jt + 1) * TQ)
                x_jt = x_tiles.pop(jt)
                if jt + 1 < NTQ:
                    emit_x(jt + 1)

                # rope rows: r1 = dims 0:8, r2 = dims 8:16 of each (tn, head)
                ps_r = sps.tile([128, 2, TQ], F32, tag="s")
                for mt in range(2):
                    for c in range(C // 128):
                        nc.tensor.matmul(
                            ps_r[:, mt, :],
                            wqk_t[:, c, 128 * mt:128 * mt + 128],
                            x_jt[:, c], start=(c == 0), stop=(c == C // 128 - 1))
                    yield
                t1 = rtp.tile([96, TQ], F32, tag="t1")
                t2 = rtp.tile([96, TQ], F32, tag="t2")
                nc.vector.tensor_tensor(t1, ps_r[0:96, 0, :], cos_t[:, ts], MUL)
                nc.vector.tensor_tensor(t2, ps_r[0:96, 1, :], sin_t[:, ts], MUL)
                nc.vector.tensor_tensor(rot1[:, ts], t1, t2, SUB)
                t3 = rtp.tile([96, TQ], F32, tag="t1")
                t4 = rtp.tile([96, TQ], F32, tag="t2")
                nc.vector.tensor_tensor(t3, ps_r[0:96, 1, :], cos_t[:, ts], MUL)
                nc.vector.tensor_tensor(t4, ps_r[0:96, 0, :], sin_t[:, ts], MUL)
                nc.vector.tensor_tensor(rot2[:, ts], t3, t4, ADD)
                yield

                # per-block full-128-row tiles: pass dims land at partitions
                # 0:48 / 64:112 straight from the matmul (wqk cols are zero
                # at 48:64 / 112:128); the rotated rope rows are accumulated
                # into 48:64 / 112:128 by one-hot routing matmuls; one DVE
                # copy evicts the finished block.  k blocks first so the
                # attention of this jt can begin as early as possible.
                def inject_evict(b, ps):
                    nc.tensor.matmul(
                        ps, rm_t[:, 256 * b:256 * b + 128], rot1[:, ts],
                        start=False, stop=False)
                    nc.tensor.matmul(
                        ps, rm_t[:, 256 * b + 128:256 * b + 256], rot2[:, ts],
                        start=False, stop=True)
                    nc.vector.tensor_copy(qk_sb[:, b, ts], ps)

                pending = None
                for b in (3, 4, 5, 0, 1, 2):
                    ps = flx.tile([128, TQ], F32, tag="flex")
                    for c in range(3):
                        nc.tensor.matmul(
                            ps, wqk_t[:, c, 256 + 128 * b:384 + 128 * b],
                            x_jt[:, c], start=(c == 0), stop=False)
                    yield
                    for c in range(3, C // 128):
                        nc.tensor.matmul(
                            ps, wqk_t[:, c, 256 + 128 * b:384 + 128 * b],
                            x_jt[:, c], start=False, stop=False)
                    if pending is not None:
                        inject_evict(*pending)
                    pending = (b, ps)
                    yield
                inject_evict(*pending)
                yield

                for vt in range(TQ // 128):
                    pvf = flx.tile([128, TQ], F32, tag="flex")
                    pv = pvf[:, 0:CL]
                    kt0 = jt * (TQ // 128) + vt
                    for c in range(C // 128):
                        nc.tensor.matmul(
                            pv, x_jt[:, c, vt * 128:(vt + 1) * 128],
                            wv_t[:, c], start=(c == 0), stop=(c == C // 128 - 1))
                    nc.vector.tensor_copy(
                        v_sb[:, kt0, :, 0:HD],
                        pv.rearrange("p (h d) -> p h d", d=HD))
                    yield

            def gen_attn(jq):
                """Scores + exp + attn@v + softmax normalization for q-tile jq.

                Software-pipelined across head pairs: pair p's attn@v matmuls
                are emitted interleaved with pair p+1's score/exp stream, so
                the PE has ready work while the ACT paces the exps (the exps
                also get a full pair of lead time before their avs)."""
                qs = slice(jq * TQ, (jq + 1) * TQ)
                nkt = 4 * (jq + 1)

                def emit_scores_exp(p, kt, qz):
                    m = kt - 4 * jq
                    a = 0 if m < 0 else 128 * m
                    ks = slice(kt * 128, (kt + 1) * 128)
                    kb = qk_sb[:, NPAIR + p, :]
                    sg = sps.tile([128, 2, TQ], F32, tag="s")
                    for h in range(2):
                        nc.tensor.matmul(
                            sg[:, h, a:TQ], kb[:, ks], qz[:, h, a:TQ],
                            start=True, stop=True)
                    ep = xpp.tile([128, 2, TQ], BF16, tag="e")
                    nc.scalar.activation(ep[:, :, a:TQ], sg[:, :, a:TQ],
                                         AF.Exp, scale=0.125)
                    if m >= 0:
                        nc.gpsimd.tensor_tensor(
                            ep[:, :, a:a + 128],
                            ep[:, :, a:a + 128], tri_t, MUL)
                    return (kt, a, ep)

                def emit_av(p, o_ps, item):
                    kt, a, ep = item
                    for h in range(2):
                        nc.tensor.matmul(
                            o_ps[:, h, a:TQ],
                            v_sb[:, kt, 2 * p + h, :], ep[:, h, a:TQ],
                            start=(kt == 0), stop=(kt == nkt - 1))

                def emit_norm(p, o_ps):
                    oun = onp.tile([128, 2, TQ], F32, tag="oun")
                    nc.vector.tensor_copy(oun[0:65, :, :], o_ps[0:65, :, :])
                    rs2 = onp.tile([2, TQ], F32, tag="rs2")
                    for h in range(2):
                        nc.sync.dma_start(rs2[h:h + 1, :], oun[64:65, h, :])
                    rinv2 = onp.tile([2, TQ], F32, tag="rinv2")
                    nc.vector.reciprocal_approx_fast(rinv2, rs2)
                    rinv2b = onp.tile([2, TQ], BF16, tag="rinv2b")
                    nc.vector.tensor_copy(rinv2b, rinv2)
                    bc = flx.tile([128, TQ], F32, tag="flex")
                    nc.tensor.matmul(bc, e2_t, rinv2b, start=True, stop=True)
                    nc.vector.tensor_tensor(
                        o_sb[0:64, p, qs], oun[0:64, 0, :], bc[0:64], MUL)
                    nc.vector.tensor_tensor(
                        o_sb[64:128, p, qs], oun[0:64, 1, :], bc[64:128], MUL)

                def emit_qz(p):
                    qz = qz0 if p % 2 == 0 else qz1
                    nc.vector.tensor_copy(qz[0:64, 0, :], qk_sb[0:64, p, qs])
                    nc.vector.tensor_copy(qz[64:128, 1, :],
                                          qk_sb[64:128, p, qs])
                    return qz

                qz_next = emit_qz(0)
                for p in range(NPAIR):
                    qz = qz_next
                    o_ps = ops.tile([128, 2, TQ], F32, tag="o")
                    held = None
                    for kt in range(nkt):
                        item = emit_scores_exp(p, kt, qz)
                        if kt == 0 and p + 1 < NPAIR:
                            # prefetch the next pair's zero-padded q copy a
                            # full pair early (its tile's last reader was
                            # pair p-1, long emitted)
                            qz_next = emit_qz(p + 1)
                        if held is not None:
                            emit_av(p, o_ps, held)
                        held = item
                        yield
                    emit_av(p, o_ps, held)
                    emit_norm(p, o_ps)
                    yield

            def gen_outproj(jq, act_evict=False):
                qs = slice(jq * TQ, (jq + 1) * TQ)
                ost = osg.tile([128, C // 128, TQ], F32, tag="ost")
                for dt in range(C // 128):
                    po = flx.tile([128, TQ], F32, tag="flex")
                    for p in range(NPAIR):
                        nc.tensor.matmul(
                            po, wo_t[:, p, dt * 128:(dt + 1) * 128],
                            o_sb[:, p, qs], start=(p == 0), stop=(p == NPAIR - 1))
                    if act_evict:
                        nc.scalar.copy(ost[:, dt, :], po)
                    else:
                        nc.vector.tensor_copy(ost[:, dt, :], po)
                    yield
                nc.sync.dma_start(
                    out.rearrange("(do p) t -> p do t", p=128)[:, :, qs], ost)

            # proj(0) alone; then attn(jt) with proj(jt+1) AND outproj(jt-1)
            # interleaved into its instruction stream (fills the PE while the
            # ACT paces the exps, and keeps the PE out of its cold-clock
            # state).  outproj(jt) itself must come strictly after attn(jt)'s
            # last norm (in-order PE would deadlock on an outproj matmul
            # emitted before its o_sb producers), so it rides in the NEXT
            # tile's fill; only outproj(3) trails at the very end.
            import itertools

            for _ in gen_proj(0):
                pass
            for jt in range(NTQ):
                fills = []
                if jt + 1 < NTQ:
                    fills.append(gen_proj(jt + 1))
                if jt >= 1:
                    fills.append(gen_outproj(jt - 1))
                fill = itertools.chain(*fills)
                # pace the fill proportionally over the attention stream so
                # it does not exhaust before the exp-paced end of the tile
                main_n = 3 * (4 * (jt + 1) + 1)
                fill_n = (21 if jt + 1 < NTQ else 0) + (6 if jt >= 1 else 0)
                # last tile: hold its few fill units for the exp-paced tail
                i0 = (2 * main_n) // 3 if jt + 1 >= NTQ else 0
                done = 0
                for i, _ in enumerate(gen_attn(jt)):
                    if i < i0:
                        continue
                    while done * (main_n - i0) < (i - i0 + 1) * fill_n:
                        next(fill, None)
                        done += 1
                for _ in fill:
                    pass
            for _ in gen_outproj(NTQ - 1, act_evict=True):
                pass

            if debug:
                with tc.tile_pool(name="dbgp", bufs=1) as dbp:
                    for blk in range(2 * NPAIR):
                        for ch in range(NTQ):
                            cs = slice(ch * TQ, (ch + 1) * TQ)
                            dcp = dbp.tile([128, TQ], F32, tag="dbgc")
                            nc.scalar.copy(dcp, qk_sb[:, blk, cs])
                            nc.sync.dma_start(dbg_qk[:, blk, cs], dcp)
                    for p in range(NPAIR):
                        for ch in range(NTQ):
                            cs = slice(ch * TQ, (ch + 1) * TQ)
                            dcp = dbp.tile([128, TQ], F32, tag="dbgc")
                            nc.scalar.copy(dcp, o_sb[:, p, cs])
                            nc.sync.dma_start(dbg_o[:, p, cs], dcp)
                    for kt in range(NKT):
                        dcv = dbp.tile([128, NHL * 128], F32, tag="dbgv")
                        nc.scalar.copy(
                            dcv, v_sb[:, kt].rearrange("p b c -> p (b c)"))
                        nc.sync.dma_start(
                            dbg_v[:, kt].rearrange("p b c -> p (b c)"), dcv)

    nc.compile()
    return nc


def _host_inputs(x, w_qkv, w_out):
    """Build per-core input dicts. Core i: batch i//2, head-group i%2."""
    import ml_dtypes

    xf = np.asarray(x, dtype=np.float32).astype(ml_dtypes.bfloat16)
    w3 = np.asarray(w_qkv, dtype=np.float32).reshape(3, NH, HD, C)
    wo = np.asarray(w_out, dtype=np.float32)

    per_group = []
    for g in range(2):
        h0 = g * NHL
        zpad = np.zeros((32, C), dtype=np.float32)
        rows = []
        for dd0 in (0, 8):                           # r1 cols, then r2 cols
            for hlp in range(2):
                for tn in range(2):
                    for pr in range(NPAIR):
                        rows.append(
                            w3[tn, h0 + 2 * pr + hlp, dd0:dd0 + 8])  # [8, C]
            rows.append(zpad)                        # pad M-tile to 128
        zpad16 = np.zeros((16, C), dtype=np.float32)
        for tn in range(2):
            for pr in range(NPAIR):
                for hlp in range(2):
                    rows.append(w3[tn, h0 + 2 * pr + hlp, 16:64])  # [48, C]
                    rows.append(zpad16)              # rope rows 48:64/112:128
        wqk = np.concatenate(rows, axis=0)                  # [1024, C]
        wqkt = np.ascontiguousarray(wqk.T.astype(ml_dtypes.bfloat16))
        wv = w3[2, h0:h0 + NHL].reshape(CL, C)              # [384, C]
        wvt = np.ascontiguousarray(wv.T.astype(ml_dtypes.bfloat16))
        wotr = np.ascontiguousarray(
            wo[:, g * CL:(g + 1) * CL].T.astype(ml_dtypes.bfloat16))
        per_group.append((wqkt, wvt, wotr))

    j = np.arange(RD // 2, dtype=np.float64)
    freqs = 1.0 / (10000.0 ** (2 * j / RD))
    t = np.arange(T, dtype=np.float64)
    ang = t[None, :] * freqs[:, None]                        # [8, T]
    # rot row order (hlp, tnpr, rr): freq row = rr = row % 8
    cosb = np.ascontiguousarray(
        np.tile(np.cos(ang), (12, 1)).astype(ml_dtypes.bfloat16))
    sinb = np.ascontiguousarray(
        np.tile(np.sin(ang), (12, 1)).astype(ml_dtypes.bfloat16))

    kk = np.arange(128)[:, None]
    qq = np.arange(128)[None, :]
    tri = np.tile((kk <= qq).astype(ml_dtypes.bfloat16), (1, 2))
    rm = np.zeros((96, 12, 128), dtype=np.float32)
    for hlp in range(2):
        for b in range(6):
            for r in range(8):
                k = 48 * hlp + 8 * b + r
                rm[k, 2 * b, 64 * hlp + 48 + r] = 1.0
                rm[k, 2 * b + 1, 64 * hlp + 56 + r] = 1.0
    rmats = np.ascontiguousarray(
        rm.reshape(96, 12 * 128).astype(ml_dtypes.bfloat16))
    e2 = np.zeros((2, 128), dtype=ml_dtypes.bfloat16)
    e2[0, 0:64] = 1.0
    e2[1, 64:128] = 1.0
    in_maps = []
    for i in range(8):
        b, g = divmod(i, 2)
        wqkt, wvt, wotr = per_group[g]
        in_maps.append({
            "xt": np.ascontiguousarray(xf[b].T),
            "wqkt": wqkt, "wvt": wvt, "wot": wotr,
            "cosb": cosb, "sinb": sinb, "tri": tri, "e2": e2,
            "rmats": rmats,
        })
    return in_maps


def kernel(x, w_qkv, w_out, _trace=False):
    from concourse.bass_utils import run_bass_kernel_spmd

    if "nc" not in _cache:
        _cache["nc"] = _build()
    nc = _cache["nc"]
    in_maps = _host_inputs(x, w_qkv, w_out)
    res = run_bass_kernel_spmd(nc, in_maps, core_ids=list(range(8)),
                               trace=_trace)
    _cache["last_result"] = res
    out = np.empty((B, T, C), dtype=np.float32)
    for b in range(B):
        acc = res.results[2 * b]["out"].astype(np.float32) + \
            res.results[2 * b + 1]["out"].astype(np.float32)
        out[b] = acc.T
    return out
